# revision 17
# baseline (speedup 1.0000x reference)
"""DigitCaps routing kernel for 8 Trainium2 NeuronCores.

Strategy: shard the input-capsule axis I=1152 as 144 per core (9 SBUF tiles of
128 = 16 i x 8 j). Per routing iteration each core computes its partial
s[b,o,d] = sum_{i local,j} exp(b_ij)[i,o] * W[i,o,d,j] * x[b,i,j] with K=128
matmuls, then one AllReduce fuses the s-partials with the softmax-denominator
partials (row 512 of the payload). The softmax normalization is folded into
squash. The agreement update is computed from G = sum_b x (x) v (PSUM
accumulation over the full batch), one elementwise multiply with W, a free-dim
reduction over d, and an indicator matmul that reduces over j on the PE.

All small matmuls are batched into wide-N matmuls (N=480) to minimize
PE->PSUM->DVE handoffs, which dominate the serial chain cost on this part.
"""
import numpy as np

import concourse.bacc as bacc
import concourse.mybir as mybir
import concourse.tile as tile
from concourse.bass_utils import run_bass_kernel_spmd

N_CORES = 8
B, I, O, D, J = 512, 1152, 10, 16, 8
IL = I // N_CORES          # 144 local input capsules
G = IL * J // 128          # 9 ij tiles of 128 partitions
M = B // 128               # 4 batch chunks
C = 3                      # g-chunks of 3 tiles -> N=480 wide ops
OD = O * D                 # 160
GO = G * O                 # 90
W3 = 3 * OD                # 480
PAY = M * OD + 16          # AR payload width: 640 s + 16 den/pad cols
NIT = 3
F32 = mybir.dt.float32
Act = mybir.ActivationFunctionType
Alu = mybir.AluOpType

_cache = {}


def _build(repeat=1, no_ar=False, skip_wc=False, skip_agree=False, warm=0, cc_warm=False):
    nc = bacc.Bacc("TRN2", target_bir_lowering=False, debug=False, num_devices=N_CORES)
    xT_e = nc.dram_tensor("xT", [IL * J, B], F32, kind="ExternalInput")
    xN_e = nc.dram_tensor("xN", [B, IL * J], F32, kind="ExternalInput")
    w2_e = nc.dram_tensor("w2", [IL * J, OD], F32, kind="ExternalInput")
    ind_e = nc.dram_tensor("ind", [16, 128], F32, kind="ExternalInput")
    indj_e = nc.dram_tensor("indj", [128, 16], F32, kind="ExternalInput")
    v_e = nc.dram_tensor("v_out", [16, M * OD], F32, kind="ExternalOutput")

    with tile.TileContext(nc) as tc:
        with (
            tc.tile_pool(name="const", bufs=1) as constp,
            tc.tile_pool(name="big", bufs=1) as big,
            tc.tile_pool(name="work", bufs=2) as work,
            tc.tile_pool(name="ps_s", bufs=2, space="PSUM") as ps_s_pool,
            tc.tile_pool(name="ps_g", bufs=3, space="PSUM") as ps_g_pool,
            tc.tile_pool(name="ps_c", bufs=1, space="PSUM") as ps_c_pool,
            tc.tile_pool(name="ps_m", bufs=1, space="PSUM") as ps_m_pool,
            tc.tile_pool(name="dram", bufs=2, space="DRAM") as dram,
        ):
            # ---- persistent inputs ----
            # interleave w2/xT per-g so the first s-matmul group can start
            # after ~2 chunks land; xN is only needed ~25us in (G4 phase).
            xT = big.tile([128, G * B], F32)        # [p=(i16,j8), (g, b)]
            w2 = big.tile([128, G * OD], F32)       # [p=(i16,j8), (g, o, d)]
            xN = big.tile([128, M * IL * J], F32)   # [p=b, (m, ij)]
            for g in range(G):
                nc.sync.dma_start(out=w2[:, g * OD:(g + 1) * OD],
                                  in_=w2_e[g * 128:(g + 1) * 128, :])
                nc.sync.dma_start(out=xT[:, g * B:(g + 1) * B],
                                  in_=xT_e[g * 128:(g + 1) * 128, :])
            for m in range(M):
                nc.sync.dma_start(out=xN[:, m * IL * J:(m + 1) * IL * J],
                                  in_=xN_e[m * 128:(m + 1) * 128, :])
            ind = constp.tile([16, 128], F32)
            nc.sync.dma_start(out=ind[:], in_=ind_e[:])
            indj = constp.tile([128, 16], F32)
            nc.sync.dma_start(out=indj[:], in_=indj_e[:])
            ones1 = constp.tile([1, 128], F32)
            nc.vector.memset(ones1[:], 1.0)
            ones16 = constp.tile([16, 1], F32)
            nc.vector.memset(ones16[:], 1.0)

            denrow = constp.tile([1, OD], F32)      # AR payload row 512
            nc.vector.memset(denrow[:], 0.0)
            # iteration 1: b=0 -> e=1 -> per-core denominator partial = IL
            nc.vector.memset(denrow[0:1, 0:O], float(IL))

            for rep in range(repeat):
              b_tiles = [big.tile([16, GO], F32, name=f"bstate{rep}_{i}")
                         for i in range(NIT - 1)]
              for t in range(NIT):
                last = t == NIT - 1
                if t > 0 and not skip_agree:
                    # e = exp(b); local denominator partial
                    e_sb = work.tile([16, GO], F32)
                    nc.scalar.activation(e_sb[:], b_tiles[t - 1][:], Act.Exp)
                    actd = work.tile([1, 1], F32, name="actd")
                    nc.scalar.activation(actd[:], ones1[0:1, 0:1], Act.Sqrt)
                    ps_misc = ps_m_pool.tile([128, 512], F32,
                                             name="ps_misc", tag="misc")
                    nc.tensor.matmul(ps_misc[0:1, 96:96 + GO], ones16[:], e_sb[:],
                                     start=True, stop=True)
                    dview = ps_misc[0:1, 96:96 + GO].rearrange(
                        "p (g o) -> p o g", g=G)
                    nc.vector.reduce_sum(denrow[0:1, 0:O], dview,
                                         axis=mybir.AxisListType.X)
                    # Wc = w2 * broadcast(e): 3 wide MMs (N=480) + 3 wide TTs
                    wc = work.tile([128, G * OD], F32)
                    for c in range(C):
                        ps_ce = ps_c_pool.tile([128, W3], F32)
                        rhs = e_sb[:, c * 30:(c + 1) * 30] \
                            .rearrange("p (g o) -> p g o", g=3) \
                            .unsqueeze(3).broadcast_to([16, 3, O, D])
                        nc.tensor.matmul(ps_ce[:], ind[:], rhs,
                                         start=True, stop=True)
                        nc.vector.tensor_tensor(
                            wc[:, c * W3:(c + 1) * W3],
                            w2[:, c * W3:(c + 1) * W3], ps_ce[:], op=Alu.mult)
                    s_rhs = w2 if skip_wc else wc
                else:
                    s_rhs = w2

                # partial s: [b, od] accumulated over the 9 local ij tiles.
                # AR payload = [128, 656]: cols 0:640 s-chunks, col 640+ den
                # (nonzero only on partition 0 -- other partitions add zeros).
                HALF = 2 * OD  # cols 0:320 | 320:656
                ar_in = dram.tile([128, PAY], F32, name="ar_in")
                ar_out = dram.tile([128, PAY], F32, name="ar_out")
                s_stage = work.tile([128, PAY], F32)
                nc.vector.memset(s_stage[:, M * OD:PAY], 0.0)
                if t == 0:
                    nc.vector.memset(s_stage[0:1, M * OD:M * OD + O], float(IL))
                elif not last:
                    nc.vector.tensor_copy(s_stage[0:1, M * OD:M * OD + O],
                                          denrow[0:1, 0:O])
                else:
                    # ReduceScatter path: every core must receive den, so
                    # replicate the partial onto all 128 partitions
                    ps_db = ps_m_pool.tile([128, 512], F32, name="ps_db",
                                           tag="misc")
                    nc.tensor.matmul(ps_db[:, 0:O], ones1[:], denrow[0:1, 0:O],
                                     start=True, stop=True)
                    nc.vector.tensor_copy(s_stage[:, M * OD:M * OD + O],
                                          ps_db[:, 0:O])
                for m in range(M):
                    ps_s = ps_s_pool.tile([128, OD], F32)
                    for g in range(G):
                        nc.tensor.matmul(
                            ps_s[:],
                            xT[:, g * B + m * 128: g * B + (m + 1) * 128],
                            s_rhs[:, g * OD:(g + 1) * OD],
                            start=(g == 0), stop=(g == G - 1))
                    nc.scalar.activation(s_stage[:, m * OD:(m + 1) * OD], ps_s[:],
                                         Act.Copy)
                    # stream each chunk out while the next one computes
                    nc.sync.dma_start(out=ar_in[:, m * OD:(m + 1) * OD],
                                      in_=s_stage[:, m * OD:(m + 1) * OD])
                nc.sync.dma_start(out=ar_in[:, M * OD:PAY],
                                  in_=s_stage[:, M * OD:PAY])
                if warm and not last:
                    # keep the PE pstate hot through the AR window
                    ps_j = ps_c_pool.tile([128, W3], F32, name="ps_j", tag="ps_ce")
                    for w in range(warm):
                        nc.tensor.matmul(ps_j[:], xT[:, 0:128],
                                         xT[:, 128:128 + W3],
                                         start=(w == 0), stop=(w == warm - 1))
                if last:
                    rs_out = dram.tile([16, PAY], F32, name="rs_out")
                    if no_ar:
                        nc.sync.dma_start(out=rs_out[:, :], in_=ar_in[0:16, :])
                    else:
                        nc.gpsimd.collective_compute(
                            "ReduceScatter", Alu.add,
                            replica_groups=[list(range(N_CORES))],
                            ins=[ar_in.opt()], outs=[rs_out.opt()])
                    sl_sb = work.tile([16, PAY], F32)
                    nc.sync.dma_start(out=sl_sb[:, :], in_=rs_out[:, :])
                    # squash on the local 16-partition slice; den already on
                    # every partition, so no PE broadcast needed
                    ivq = work.tile([16, 32], F32)
                    nc.vector.reciprocal(ivq[:, 0:O],
                                         sl_sb[:, M * OD:M * OD + O])
                    nc.vector.tensor_tensor(ivq[:, 16:16 + O], ivq[:, 0:O],
                                            ivq[:, 0:O], op=Alu.mult)
                    sqr2 = work.tile([16, M * OD], F32)
                    nc.scalar.activation(sqr2[:], sl_sb[:, 0:M * OD], Act.Square)
                    sqs2 = work.tile([16, M * O], F32)
                    nc.vector.reduce_sum(
                        sqs2[:],
                        sqr2[:].rearrange("p (m o d) -> p m o d", m=M, o=O),
                        axis=mybir.AxisListType.X)
                    sqt2 = work.tile([16, M * O], F32)
                    nc.vector.tensor_tensor(
                        sqt2[:].rearrange("p (m o) -> p m o", m=M),
                        sqs2[:].rearrange("p (m o) -> p m o", m=M),
                        ivq[:, 16:16 + O].unsqueeze(1).broadcast_to([16, M, O]),
                        op=Alu.mult)
                    rt2 = work.tile([16, M * O], F32)
                    nc.scalar.activation(rt2[:], sqt2[:], Act.Sqrt)
                    d22 = work.tile([16, M * O], F32)
                    nc.vector.tensor_scalar_add(d22[:], sqt2[:], 1.0)
                    rc2 = work.tile([16, M * O], F32)
                    nc.vector.reciprocal(rc2[:], d22[:])
                    gfa = work.tile([16, M * O], F32)
                    nc.vector.tensor_tensor(gfa[:], rt2[:], rc2[:], op=Alu.mult)
                    gfb = work.tile([16, M * O], F32)
                    nc.vector.tensor_tensor(
                        gfb[:].rearrange("p (m o) -> p m o", m=M),
                        gfa[:].rearrange("p (m o) -> p m o", m=M),
                        ivq[:, 0:O].unsqueeze(1).broadcast_to([16, M, O]),
                        op=Alu.mult)
                    vsl = work.tile([16, M * OD], F32)
                    nc.vector.tensor_tensor(
                        vsl[:].rearrange("p (m o d) -> p m o d", m=M, o=O),
                        sl_sb[:, 0:M * OD].rearrange("p (m o d) -> p m o d",
                                                     m=M, o=O),
                        gfb[:].rearrange("p (m o) -> p m o", m=M).unsqueeze(3)
                        .broadcast_to([16, M, O, D]),
                        op=Alu.mult)
                    nc.sync.dma_start(out=v_e[:, :], in_=vsl[:])
                    continue

                if no_ar:
                    nc.sync.dma_start(out=ar_out[:, :], in_=ar_in[:, :])
                else:
                    if cc_warm:
                        dum_i = dram.tile([1, 16], F32, name="dum_i")
                        dum_o = dram.tile([1, 16], F32, name="dum_o")
                        nc.sync.dma_start(out=dum_i[:], in_=s_stage[0:1, 0:16])
                        nc.gpsimd.collective_compute(
                            "AllReduce", Alu.add,
                            replica_groups=[list(range(N_CORES))],
                            ins=[dum_i.opt()], outs=[dum_o.opt()])
                    nc.gpsimd.collective_compute(
                        "AllReduce", Alu.add,
                        replica_groups=[list(range(N_CORES))],
                        ins=[ar_in.opt()], outs=[ar_out.opt()])
                s_sb = work.tile([128, PAY], F32)
                nc.sync.dma_start(out=s_sb[:, :], in_=ar_out[:, :])
                ivp = work.tile([1, 32], F32)
                nc.vector.reciprocal(ivp[0:1, 0:O], s_sb[0:1, M * OD:M * OD + O])
                nc.vector.tensor_tensor(ivp[0:1, 16:16 + O], ivp[0:1, 0:O],
                                        ivp[0:1, 0:O], op=Alu.mult)
                ps_bc = ps_m_pool.tile([128, 512], F32, name="ps_bc", tag="misc")
                nc.tensor.matmul(ps_bc[:, 0:32], ones1[:], ivp[:],
                                 start=True, stop=True)
                iv1 = ps_bc[:, 0:O]        # invden broadcast [128, 10]
                iv2 = ps_bc[:, 16:16 + O]  # invden^2 broadcast [128, 10]

                # squash with folded normalization:
                # v = s_raw * invd * sqrt(sq)/(1+sq),  sq = invd^2 * sum_d s_raw^2
                sqr = work.tile([128, M * OD], F32)
                nc.scalar.activation(sqr[:], s_sb[:, 0:M * OD], Act.Square)
                sqs = work.tile([128, M * O], F32)
                nc.vector.reduce_sum(
                    sqs[:], sqr[:].rearrange("p (m o d) -> p m o d", m=M, o=O),
                    axis=mybir.AxisListType.X)
                sqt = work.tile([128, M * O], F32)
                nc.vector.tensor_tensor(
                    sqt[:].rearrange("p (m o) -> p m o", m=M),
                    sqs[:].rearrange("p (m o) -> p m o", m=M),
                    iv2.unsqueeze(1).broadcast_to([128, M, O]), op=Alu.mult)
                rt = work.tile([128, M * O], F32)
                nc.scalar.activation(rt[:], sqt[:], Act.Sqrt)
                if not last:
                    actd2 = work.tile([1, 1], F32, name="actd2")
                    nc.scalar.activation(actd2[:], ones1[0:1, 0:1], Act.Exp)
                d2 = work.tile([128, M * O], F32)
                nc.vector.tensor_scalar_add(d2[:], sqt[:], 1.0)
                rc = work.tile([128, M * O], F32)
                nc.vector.reciprocal(rc[:], d2[:])
                gf = work.tile([128, M * O], F32)
                nc.vector.tensor_tensor(gf[:], rt[:], rc[:], op=Alu.mult)
                gf2 = work.tile([128, M * O], F32)
                nc.vector.tensor_tensor(
                    gf2[:].rearrange("p (m o) -> p m o", m=M),
                    gf[:].rearrange("p (m o) -> p m o", m=M),
                    iv1.unsqueeze(1).broadcast_to([128, M, O]), op=Alu.mult)
                v_sb = work.tile([128, M * OD], F32)
                nc.vector.tensor_tensor(
                    v_sb[:].rearrange("p (m o d) -> p m o d", m=M, o=O),
                    s_sb[:, 0:M * OD].rearrange("p (m o d) -> p m o d", m=M, o=O),
                    gf2[:].rearrange("p (m o) -> p m o", m=M).unsqueeze(3)
                    .broadcast_to([128, M, O, D]),
                    op=Alu.mult)

                if skip_agree:
                    continue

                # G4 = sum_b x (x) v; P4 = w2*G4; reduce d; reduce j on PE
                p4 = work.tile([128, G * OD], F32)
                p4d = work.tile([128, GO], F32)
                ps_b = ps_m_pool.tile([16, GO], F32, name="ps_b", tag="psb")
                for g in range(G):
                    ps_g = ps_g_pool.tile([128, OD], F32)
                    for m in range(M):
                        nc.tensor.matmul(
                            ps_g[:],
                            xN[:, m * IL * J + g * 128: m * IL * J + (g + 1) * 128],
                            v_sb[:, m * OD:(m + 1) * OD],
                            start=(m == 0), stop=(m == M - 1))
                    nc.vector.tensor_tensor(
                        p4[:, g * OD:(g + 1) * OD],
                        w2[:, g * OD:(g + 1) * OD], ps_g[:], op=Alu.mult)
                    # reduce d for this tile immediately (overlaps next MMs)
                    nc.vector.reduce_sum(
                        p4d[:, g * O:(g + 1) * O],
                        p4[:, g * OD:(g + 1) * OD].rearrange(
                            "p (o d) -> p o d", o=O),
                        axis=mybir.AxisListType.X)
                for c in range(C):
                    nc.tensor.matmul(ps_b[:, c * 30:(c + 1) * 30], indj[:],
                                     p4d[:, c * 30:(c + 1) * 30],
                                     start=True, stop=True)
                if t == 0:
                    nc.vector.tensor_copy(b_tiles[0][:], ps_b[:])
                else:
                    nc.vector.tensor_tensor(b_tiles[t][:], b_tiles[t - 1][:],
                                            ps_b[:], op=Alu.add)

    nc.compile()
    return nc


def _host_inputs(x, W):
    """Slice + lay out per-core inputs."""
    x = np.ascontiguousarray(x, dtype=np.float32)
    W = np.ascontiguousarray(W, dtype=np.float32)
    ind = np.zeros((16, 128), dtype=np.float32)
    for k in range(16):
        ind[k, k * 8:(k + 1) * 8] = 1.0
    indj = np.ascontiguousarray(ind.T) / float(B)
    in_maps = []
    for c in range(N_CORES):
        sl = slice(c * IL, (c + 1) * IL)
        xs = x[:, sl, :].reshape(B, IL * J)
        ws = W[sl]  # [IL, O, D, J]
        in_maps.append({
            "xT": np.ascontiguousarray(xs.T),
            "xN": np.ascontiguousarray(xs),
            "w2": np.ascontiguousarray(
                ws.transpose(0, 3, 1, 2).reshape(IL * J, OD)),
            "ind": ind,
            "indj": indj,
        })
    return in_maps


def kernel(x, W):
    if "nc" not in _cache:
        _cache["nc"] = _build()
    nc = _cache["nc"]
    in_maps = _host_inputs(x, W)
    res = run_bass_kernel_spmd(nc, in_maps, list(range(N_CORES)))
    # reassemble: ReduceScatter gave core k partitions [16k, 16k+16);
    # per-core output is [16, (m, od)] with global b = 128*m + 16*k + p
    v = np.empty((B, OD), dtype=np.float32)
    for k in range(N_CORES):
        vk = res.results[k]["v_out"].reshape(16, M, OD)
        for m in range(M):
            v[128 * m + 16 * k:128 * m + 16 * k + 16, :] = vk[:, m, :]
    return v.reshape(B, O, D, 1).astype(np.float32)
